# revision 5
# baseline (speedup 1.0000x reference)
"""KNN-graph (K=2) adjacency kernel for Trainium2, 8 NeuronCores SPMD.

Host<->device traffic over the tunnel is the bottleneck (~65 MB/s), so the
kernel never moves the dense [N, N] adjacency (1 GiB) or a replicated X^T
(32 MiB). Each core uploads only its own [64, 2048] X^T slice (4.25 MiB
total); a first XLA jit all-gathers the slices on-device into a replicated
[64, 16384] X^T, and a second jit runs the Bass NEFF with device-resident
operands. The NEFF returns just the per-row nearest-neighbor index
([128, 16] per core); the dense adjacency is assembled host-side, with the
zero-fill + self-diagonal overlapped with the device pipeline.

Per core (2048 query rows, 16 blocks of 128):
  value[i, j] = 2*<x_i, x_j> - sq_i - sq_j   (= -dist2, one matmul with
  contraction 66 = 64 features + two augmented rows), in GLOBAL column
  order. Self-distance exclusion falls out of taking the SECOND-largest
  value per row: the largest is always self at ~0, every other point is at
  -dist2 <= -20 for randn data. Per 2048-col window: top-8 (vector.max);
  global top-8 of the 64 window candidates; per-window MaxIndex of the
  global 2nd-best, combined across windows by min over (index + offset).
"""

import os
import sys
import functools

import numpy as np

for _p in ("/opt/trn_rl_repo",):
    if _p not in sys.path and os.path.isdir(_p):
        sys.path.insert(0, _p)

N = 16384
D = 64
NCORES = 8
RPC = N // NCORES          # rows per core = 2048
P = 128                    # partitions / rows per block
NBLK = RPC // P            # 16 blocks per core
NCHUNK = N // 512          # 32 matmul chunks per block
WIN = 2048                 # fold window (4 chunks)


def _body(nc, tc, tile, bass, mybir, out, xtp, qt):
    from contextlib import ExitStack

    f32 = mybir.dt.float32
    u32 = mybir.dt.uint32
    AL = mybir.AluOpType
    AF = mybir.ActivationFunctionType
    X_AX = mybir.AxisListType.X

    ctx = ExitStack()
    with ctx:
        const = ctx.enter_context(tc.tile_pool(name="const", bufs=1))
        aug = ctx.enter_context(tc.tile_pool(name="aug", bufs=1))
        sqp = ctx.enter_context(tc.tile_pool(name="sqp", bufs=3))
        tmps = ctx.enter_context(tc.tile_pool(name="tmps", bufs=4))
        h1p = ctx.enter_context(tc.tile_pool(name="h1p", bufs=10))
        smalls = ctx.enter_context(tc.tile_pool(name="smalls", bufs=2))
        psum = ctx.enter_context(tc.tile_pool(name="psum", bufs=6, space="PSUM"))
        psq = ctx.enter_context(tc.tile_pool(name="psq", bufs=2, space="PSUM"))

        # ---------------- constants ----------------
        ones64 = const.tile([64, 1], f32)
        nc.vector.memset(ones64[:, :], 1.0)

        # per-row neighbor index accumulator: col b holds block b's answers
        jall = const.tile([P, NBLK], f32)

        # w*2048 per candidate slot, replicated down partitions (f32 exact)
        woffu = const.tile([P, 8], u32)
        nc.gpsimd.iota(woffu[:, :], pattern=[[WIN, 8]], base=0,
                       channel_multiplier=0)
        woff = const.tile([P, 8], f32)
        nc.vector.tensor_copy(woff[:, :], woffu[:, :])

        # ---------------- augmented operands ----------------
        # rhs rows 0-63 = X^T in global order (device-gathered, replicated)
        rhs = aug.tile([66, N], f32)
        nc.sync.dma_start(rhs[0:64, :], xtp[:, :])
        nc.vector.memset(rhs[64:65, :], -1.0)

        # own queries (this core's 2048 columns; their global position is
        # core-dependent, so they come from the per-core input directly)
        qsb = const.tile([D, RPC], f32)
        nc.sync.dma_start(qsb[:, :], qt[:, :])

        lhsT = aug.tile([66, RPC], f32)
        for k in range(4):
            sl = slice(k * 512, (k + 1) * 512)
            nc.scalar.activation(lhsT[0:64, sl], qsb[:, sl], AF.Copy, scale=2.0)
        # rows 64+65 both to -1.0 (base-partition must be 0/32/64/96); the sq_i
        # DMAs below then overwrite row 64 with +sq_i.
        nc.vector.memset(lhsT[64:66, :], -1.0)

        # sq_i = sum_d q_id^2 -> lhsT row 64 (4 chunks of 512)
        for t in range(4):
            sl = slice(t * 512, (t + 1) * 512)
            xsq = sqp.tile([64, 512], f32)
            nc.scalar.activation(xsq[:, :], qsb[:, sl], AF.Square)
            pq = psq.tile([1, 512], f32)
            nc.tensor.matmul(pq[:, :], lhsT=ones64[:, :], rhs=xsq[:, :],
                             start=True, stop=True)
            tq = tmps.tile([1, 512], f32)
            nc.vector.tensor_copy(tq[:, :], pq[:, :])
            nc.sync.dma_start(lhsT[64:65, sl], tq[:, :])

        # sq_j = sum_d x_jd^2 -> rhs row 65 (32 chunks of 512)
        for t in range(NCHUNK):
            sl = slice(t * 512, (t + 1) * 512)
            xsq = sqp.tile([64, 512], f32)
            nc.scalar.activation(xsq[:, :], rhs[0:64, sl], AF.Square)
            pq = psq.tile([1, 512], f32)
            nc.tensor.matmul(pq[:, :], lhsT=ones64[:, :], rhs=xsq[:, :],
                             start=True, stop=True)
            tq = tmps.tile([1, 512], f32)
            nc.vector.tensor_copy(tq[:, :], pq[:, :])
            nc.sync.dma_start(rhs[65:66, sl], tq[:, :])

        # ---------------- main loop ----------------
        NWINF = N // WIN  # 8 column windows
        for b in range(NBLK):
            lw = lhsT[:, b * P:(b + 1) * P]
            win = [h1p.tile([P, WIN], f32, tag="win", name=f"win_{b}_{w}")
                   for w in range(NWINF)]
            for t in range(NCHUNK):
                ps = psum.tile([P, 512], f32)
                nc.tensor.matmul(ps[:, :], lhsT=lw,
                                 rhs=rhs[:, t * 512:(t + 1) * 512],
                                 start=True, stop=True)
                dst = win[t // 4][:, (t % 4) * 512:(t % 4 + 1) * 512]
                nc.scalar.copy(dst, ps[:, :])

            # per-window top-8, then global top-8 of the 64 candidates.
            # Slot 0 is always self (-dist2 ~ 0 vs <= -20 for others), so the
            # neighbor is slot 1 -- no diagonal masking needed.
            m64 = smalls.tile([P, 64], f32, tag="m64")
            for w in range(NWINF):
                nc.vector.max(out=m64[:, w * 8:(w + 1) * 8], in_=win[w][:, :])
            vals8 = smalls.tile([P, 8], f32, tag="vals8")
            nc.vector.max(out=vals8[:, :], in_=m64[:, :])

            candf = smalls.tile([P, 8], f32, tag="candf")
            for w in range(NWINF):
                i8 = smalls.tile([P, 8], u32, tag=f"i8_{w % 2}",
                                 name=f"i8_{b}_{w}")
                nc.vector.max_index(i8[:, :], vals8[:, :], win[w][:, :])
                nc.vector.tensor_copy(candf[:, w:w + 1], i8[:, 1:2])
            # global index of the 2nd-best; not-found windows are ~4.29e9
            nc.vector.tensor_tensor(candf[:, :], candf[:, :], woff[:, :],
                                    op=AL.add)
            nc.vector.tensor_reduce(jall[:, b:b + 1], candf[:, :], axis=X_AX,
                                    op=AL.min)

        # single tiny DMA: all 2048 neighbor indices for this core
        nc.sync.dma_start(out[:, :], jall[:, :])


@functools.cache
def _build():
    import concourse.bass as bass
    import concourse.tile as tile
    from concourse import bacc, mybir

    nc = bacc.Bacc("TRN2", target_bir_lowering=False, debug=False,
                   num_devices=NCORES)
    # Declaration order == bass_exec operand order in the runner below.
    xtp = nc.dram_tensor("xtp", [D, N], mybir.dt.float32,
                         kind="ExternalInput").ap()
    qt = nc.dram_tensor("qt", [D, RPC], mybir.dt.float32,
                        kind="ExternalInput").ap()
    out = nc.dram_tensor("out", [P, NBLK], mybir.dt.float32,
                         kind="ExternalOutput").ap()
    with tile.TileContext(nc) as tc:
        _body(nc, tc, tile, bass, mybir, out, xtp, qt)
    nc.compile()
    return nc


@functools.cache
def _jits():
    """(gather_jit, exec_jit, row_sharding) — built once per process."""
    import jax
    from jax.sharding import Mesh, PartitionSpec, NamedSharding
    from jax.experimental.shard_map import shard_map
    from concourse import bass2jax

    nc = _build()
    bass2jax.install_neuronx_cc_hook()

    mesh = Mesh(np.asarray(jax.devices()[:NCORES]), ("core",))
    PSH, PREP = PartitionSpec("core"), PartitionSpec()

    # jit1: upload per-core [64, 2048] slices (stacked [512, 2048], sharded on
    # axis 0), all-gather + transpose on device -> replicated [64, N] X^T in
    # global order; pass the sharded slices through for jit2.
    def _gather(q):                       # q: [64, 2048] local shard
        g = jax.lax.all_gather(q, "core")             # [8, 64, 2048]
        xtp = g.transpose(1, 0, 2).reshape(D, N)      # [64, 16384]
        return xtp, q

    gather_jit = jax.jit(shard_map(_gather, mesh=mesh, in_specs=(PSH,),
                                   out_specs=(PREP, PSH), check_rep=False))

    # jit2: bass_exec with device-resident operands. Operand order must match
    # the BIR ExternalInput allocation order (xtp, qt), then the donated
    # zeroed output, then partition_id; neuronx_cc_hook requires the data
    # operands to be direct jit parameters in order.
    out_aval = jax.core.ShapedArray((P, NBLK), np.float32)

    def _exec(xtp, qt, zout):
        outs = bass2jax._bass_exec_p.bind(
            xtp, qt, zout, bass2jax.partition_id_tensor(),
            out_avals=(out_aval,),
            in_names=("xtp", "qt", "out", "partition_id"),
            out_names=("out",),
            lowering_input_output_aliases=(),
            sim_require_finite=True,
            sim_require_nnan=True,
            nc=nc,
        )
        return outs[0]

    exec_jit = jax.jit(
        shard_map(_exec, mesh=mesh, in_specs=(PREP, PSH, PSH),
                  out_specs=PSH, check_rep=False),
        donate_argnums=(2,), keep_unused=True)
    return gather_jit, exec_jit, NamedSharding(mesh, PSH)


def run(X):
    """Build+run; returns (adjacency [N,N] f32, per-row neighbor indices)."""
    import jax
    gather_jit, exec_jit, shard = _jits()
    X = np.asarray(X).astype(np.float32, copy=False)
    # stacked per-core X^T slices: [8*64, 2048], core c = rows [c*64, (c+1)*64)
    qstack = np.ascontiguousarray(
        X.reshape(NCORES, RPC, D).transpose(0, 2, 1)).reshape(NCORES * D, RPC)
    # async upload + async device pipeline ...
    q0 = jax.device_put(qstack, shard)
    z0 = jax.device_put(np.zeros((NCORES * P, NBLK), np.float32), shard)
    xtp_dev, q_dev = gather_jit(q0)
    o = exec_jit(xtp_dev, q_dev, z0)
    # ... overlapped with the adjacency zero-fill + self-diagonal on host
    rows = np.arange(N, dtype=np.int64)
    adj = np.zeros((N, N), dtype=np.float32)
    adj[rows, rows] = 1.0
    out = np.asarray(o)  # blocks: [8*128, 16]
    # per core [128, 16]: element [p, b] = neighbor of local row b*128 + p
    idx = np.concatenate(
        [out[c * P:(c + 1) * P].T.reshape(-1) for c in range(NCORES)])
    idx = np.clip(idx, 0.0, float(N - 1)).astype(np.int64)
    adj[rows, idx] = 1.0
    return adj, idx


def kernel(X):
    out, _ = run(X)
    return out.astype(np.float32, copy=False)


if __name__ == "__main__":
    rng = np.random.default_rng(0)
    X = rng.standard_normal((N, D)).astype(np.float32)
    out = kernel(X)
    print("out", out.shape, out.dtype, "row sums", out.sum(1)[:8])


# revision 6
# speedup vs baseline: 1.0865x; 1.0865x over previous
"""KNN-graph (K=2) adjacency kernel for Trainium2, 8 NeuronCores SPMD.

Host<->device traffic over the tunnel is the bottleneck (~65 MB/s), so the
kernel never moves the dense [N, N] adjacency (1 GiB) or a replicated X^T
(32 MiB). Each core uploads only its own [64, 2048] X^T slice (4.25 MiB
total); a first XLA jit all-gathers the slices on-device into a replicated
[64, 16384] X^T, and a second jit runs the Bass NEFF with device-resident
operands. The NEFF returns just the per-row nearest-neighbor index
([128, 16] per core); the dense adjacency is assembled host-side, with the
zero-fill + self-diagonal overlapped with the device pipeline.

Per core (2048 query rows, 16 blocks of 128):
  value[i, j] = 2*<x_i, x_j> - sq_i - sq_j   (= -dist2, one matmul with
  contraction 66 = 64 features + two augmented rows), in GLOBAL column
  order. Self-distance exclusion falls out of taking the SECOND-largest
  value per row: the largest is always self at ~0, every other point is at
  -dist2 <= -20 for randn data. Per 2048-col window: top-8 (vector.max);
  global top-8 of the 64 window candidates; per-window MaxIndex of the
  global 2nd-best, combined across windows by min over (index + offset).
"""

import os
import sys
import functools

import numpy as np

for _p in ("/opt/trn_rl_repo",):
    if _p not in sys.path and os.path.isdir(_p):
        sys.path.insert(0, _p)

N = 16384
D = 64
NCORES = 8
RPC = N // NCORES          # rows per core = 2048
P = 128                    # partitions / rows per block
NBLK = RPC // P            # 16 blocks per core
NCHUNK = N // 512          # 32 matmul chunks per block
WIN = 2048                 # fold window (4 chunks)


def _body(nc, tc, tile, bass, mybir, out, xtp, qt):
    from contextlib import ExitStack

    f32 = mybir.dt.float32
    u32 = mybir.dt.uint32
    AL = mybir.AluOpType
    AF = mybir.ActivationFunctionType
    X_AX = mybir.AxisListType.X

    ctx = ExitStack()
    with ctx:
        const = ctx.enter_context(tc.tile_pool(name="const", bufs=1))
        aug = ctx.enter_context(tc.tile_pool(name="aug", bufs=1))
        sqp = ctx.enter_context(tc.tile_pool(name="sqp", bufs=3))
        tmps = ctx.enter_context(tc.tile_pool(name="tmps", bufs=4))
        h1p = ctx.enter_context(tc.tile_pool(name="h1p", bufs=10))
        smalls = ctx.enter_context(tc.tile_pool(name="smalls", bufs=2))
        psum = ctx.enter_context(tc.tile_pool(name="psum", bufs=6, space="PSUM"))
        psq = ctx.enter_context(tc.tile_pool(name="psq", bufs=2, space="PSUM"))

        # ---------------- constants ----------------
        ones64 = const.tile([64, 1], f32)
        nc.vector.memset(ones64[:, :], 1.0)

        # per-row neighbor index accumulator: col b holds block b's answers
        jall = const.tile([P, NBLK], f32)

        # w*2048 per candidate slot, replicated down partitions (f32 exact)
        woffu = const.tile([P, 8], u32)
        nc.gpsimd.iota(woffu[:, :], pattern=[[WIN, 8]], base=0,
                       channel_multiplier=0)
        woff = const.tile([P, 8], f32)
        nc.vector.tensor_copy(woff[:, :], woffu[:, :])

        # ---------------- augmented operands ----------------
        # rhs rows 0-63 = X^T in global order (device-gathered, replicated)
        rhs = aug.tile([66, N], f32)
        nc.sync.dma_start(rhs[0:64, :], xtp[:, :])
        nc.vector.memset(rhs[64:65, :], -1.0)

        # own queries (this core's 2048 columns; their global position is
        # core-dependent, so they come from the per-core input directly)
        qsb = const.tile([D, RPC], f32)
        nc.sync.dma_start(qsb[:, :], qt[:, :])

        lhsT = aug.tile([66, RPC], f32)
        for k in range(4):
            sl = slice(k * 512, (k + 1) * 512)
            nc.scalar.activation(lhsT[0:64, sl], qsb[:, sl], AF.Copy, scale=2.0)
        # rows 64+65 both to -1.0 (base-partition must be 0/32/64/96); the sq_i
        # DMAs below then overwrite row 64 with +sq_i.
        nc.vector.memset(lhsT[64:66, :], -1.0)

        # sq_i = sum_d q_id^2 -> lhsT row 64 (4 chunks of 512)
        for t in range(4):
            sl = slice(t * 512, (t + 1) * 512)
            xsq = sqp.tile([64, 512], f32)
            nc.scalar.activation(xsq[:, :], qsb[:, sl], AF.Square)
            pq = psq.tile([1, 512], f32)
            nc.tensor.matmul(pq[:, :], lhsT=ones64[:, :], rhs=xsq[:, :],
                             start=True, stop=True)
            tq = tmps.tile([1, 512], f32)
            nc.vector.tensor_copy(tq[:, :], pq[:, :])
            nc.sync.dma_start(lhsT[64:65, sl], tq[:, :])

        # sq_j = sum_d x_jd^2 -> rhs row 65 (32 chunks of 512)
        for t in range(NCHUNK):
            sl = slice(t * 512, (t + 1) * 512)
            xsq = sqp.tile([64, 512], f32)
            nc.scalar.activation(xsq[:, :], rhs[0:64, sl], AF.Square)
            pq = psq.tile([1, 512], f32)
            nc.tensor.matmul(pq[:, :], lhsT=ones64[:, :], rhs=xsq[:, :],
                             start=True, stop=True)
            tq = tmps.tile([1, 512], f32)
            nc.vector.tensor_copy(tq[:, :], pq[:, :])
            nc.sync.dma_start(rhs[65:66, sl], tq[:, :])

        # ---------------- main loop ----------------
        NWINF = N // WIN  # 8 column windows
        for b in range(NBLK):
            lw = lhsT[:, b * P:(b + 1) * P]
            win = [h1p.tile([P, WIN], f32, tag="win", name=f"win_{b}_{w}")
                   for w in range(NWINF)]
            for t in range(NCHUNK):
                ps = psum.tile([P, 512], f32)
                nc.tensor.matmul(ps[:, :], lhsT=lw,
                                 rhs=rhs[:, t * 512:(t + 1) * 512],
                                 start=True, stop=True)
                dst = win[t // 4][:, (t % 4) * 512:(t % 4 + 1) * 512]
                nc.scalar.copy(dst, ps[:, :])

            # per-window top-8, then global top-8 of the 64 candidates.
            # Slot 0 is always self (-dist2 ~ 0 vs <= -20 for others), so the
            # neighbor is slot 1 -- no diagonal masking needed.
            m64 = smalls.tile([P, 64], f32, tag="m64")
            for w in range(NWINF):
                nc.vector.max(out=m64[:, w * 8:(w + 1) * 8], in_=win[w][:, :])
            vals8 = smalls.tile([P, 8], f32, tag="vals8")
            nc.vector.max(out=vals8[:, :], in_=m64[:, :])

            candf = smalls.tile([P, 8], f32, tag="candf")
            for w in range(NWINF):
                i8 = smalls.tile([P, 8], u32, tag=f"i8_{w % 2}",
                                 name=f"i8_{b}_{w}")
                nc.vector.max_index(i8[:, :], vals8[:, :], win[w][:, :])
                nc.vector.tensor_copy(candf[:, w:w + 1], i8[:, 1:2])
            # global index of the 2nd-best; not-found windows are ~4.29e9
            nc.vector.tensor_tensor(candf[:, :], candf[:, :], woff[:, :],
                                    op=AL.add)
            nc.vector.tensor_reduce(jall[:, b:b + 1], candf[:, :], axis=X_AX,
                                    op=AL.min)

        # single tiny DMA: all 2048 neighbor indices for this core
        nc.sync.dma_start(out[:, :], jall[:, :])


@functools.cache
def _build():
    import concourse.bass as bass
    import concourse.tile as tile
    from concourse import bacc, mybir

    nc = bacc.Bacc("TRN2", target_bir_lowering=False, debug=False,
                   num_devices=NCORES)
    # Declaration order == bass_exec operand order in the runner below.
    xtp = nc.dram_tensor("xtp", [D, N], mybir.dt.float32,
                         kind="ExternalInput").ap()
    qt = nc.dram_tensor("qt", [D, RPC], mybir.dt.float32,
                        kind="ExternalInput").ap()
    out = nc.dram_tensor("out", [P, NBLK], mybir.dt.float32,
                         kind="ExternalOutput").ap()
    with tile.TileContext(nc) as tc:
        _body(nc, tc, tile, bass, mybir, out, xtp, qt)
    nc.compile()
    return nc


@functools.cache
def _jits():
    """(gather_jit, exec_jit, row_sharding) — built once per process."""
    import jax
    from jax.sharding import Mesh, PartitionSpec, NamedSharding
    from jax.experimental.shard_map import shard_map
    from concourse import bass2jax

    nc = _build()
    bass2jax.install_neuronx_cc_hook()

    mesh = Mesh(np.asarray(jax.devices()[:NCORES]), ("core",))
    PSH, PREP = PartitionSpec("core"), PartitionSpec()

    # jit1: upload per-core [64, 2048] slices (stacked [512, 2048], sharded on
    # axis 0), all-gather + transpose on device -> replicated [64, N] X^T in
    # global order; pass the sharded slices through for jit2.
    def _gather(q):                       # q: [64, 2048] local shard
        g = jax.lax.all_gather(q, "core")             # [8, 64, 2048]
        xtp = g.transpose(1, 0, 2).reshape(D, N)      # [64, 16384]
        return xtp, q

    gather_jit = jax.jit(shard_map(_gather, mesh=mesh, in_specs=(PSH,),
                                   out_specs=(PREP, PSH), check_rep=False))

    # jit2: bass_exec with device-resident operands. Operand order must match
    # the BIR ExternalInput allocation order (xtp, qt), then the donated
    # zeroed output, then partition_id; neuronx_cc_hook requires the data
    # operands to be direct jit parameters in order.
    out_aval = jax.core.ShapedArray((P, NBLK), np.float32)

    def _exec(xtp, qt, zout):
        outs = bass2jax._bass_exec_p.bind(
            xtp, qt, zout, bass2jax.partition_id_tensor(),
            out_avals=(out_aval,),
            in_names=("xtp", "qt", "out", "partition_id"),
            out_names=("out",),
            lowering_input_output_aliases=(),
            sim_require_finite=True,
            sim_require_nnan=True,
            nc=nc,
        )
        return outs[0]

    exec_jit = jax.jit(
        shard_map(_exec, mesh=mesh, in_specs=(PREP, PSH, PSH),
                  out_specs=PSH, check_rep=False),
        donate_argnums=(2,), keep_unused=True)
    return gather_jit, exec_jit, NamedSharding(mesh, PSH)


def run(X):
    """Build+run; returns (adjacency [N,N] f32, per-row neighbor indices)."""
    import jax
    gather_jit, exec_jit, shard = _jits()
    X = np.asarray(X).astype(np.float32, copy=False)
    # stacked per-core X^T slices: [8*64, 2048], core c = rows [c*64, (c+1)*64)
    qstack = np.ascontiguousarray(
        X.reshape(NCORES, RPC, D).transpose(0, 2, 1)).reshape(NCORES * D, RPC)
    # async upload + async device pipeline ...
    q0 = jax.device_put(qstack, shard)
    z0 = jax.device_put(np.zeros((NCORES * P, NBLK), np.float32), shard)
    xtp_dev, q_dev = gather_jit(q0)
    o = exec_jit(xtp_dev, q_dev, z0)
    try:
        o.copy_to_host_async()  # stream the result back during host assembly
    except Exception:
        pass
    # ... overlapped with the adjacency zero-fill + self-diagonal on host
    rows = np.arange(N, dtype=np.int64)
    adj = np.zeros((N, N), dtype=np.float32)
    adj[rows, rows] = 1.0
    out = np.asarray(o)  # blocks: [8*128, 16]
    # per core [128, 16]: element [p, b] = neighbor of local row b*128 + p
    idx = np.concatenate(
        [out[c * P:(c + 1) * P].T.reshape(-1) for c in range(NCORES)])
    idx = np.clip(idx, 0.0, float(N - 1)).astype(np.int64)
    adj[rows, idx] = 1.0
    return adj, idx


def kernel(X):
    out, _ = run(X)
    return out.astype(np.float32, copy=False)


if __name__ == "__main__":
    rng = np.random.default_rng(0)
    X = rng.standard_normal((N, D)).astype(np.float32)
    out = kernel(X)
    print("out", out.shape, out.dtype, "row sums", out.sum(1)[:8])
